# Initial kernel scaffold
#
"""Multihead attention (B=4, S=2048, D=1024, H=16) on 8 Trainium2 NeuronCores.

Sharding: data-parallel over batch (4) x tensor-parallel over heads (2 groups
of 8 heads). Core c handles batch c//2, head-group c%2. Q/K/V projections are
column-parallel (each core owns 512 rows of Wq/Wk/Wv), attention is fully
local per head, out-projection is row-parallel (each core owns 512 columns of
Wo) producing a partial [S, D] output; the two partials per batch are summed
on the host (the "all-reduce").

Device layout (per core, all bf16 unless noted):
  xqT/xkT/xvT [1024, 2048]  = x[b].T            (host-transposed, bf16)
  wqT/wkT/wvT [1024, 512]   = W[g*512:,:].T
  woT         [512, 1024]   = Wo[:, g*512:].T
  bq/bk [128, 4] f32 (partition-major), bv [1, 512] f32
  outp        [2048, 1024] f32 (partial output)

On-chip pipeline (K proj -> V proj -> attention with Q proj interleaved):
  QT/KT [512, 2048] = W^T-slice .T @ x^T   (PE, accumulate over 8 K-chunks)
  V_aug [2048, 8*65]: V projected directly in [s, dv] layout (x^T chunks as
  the stationary operand) + a ones column per head
  per head h, per q-half (2 x 512 q):
    S^T tile [128 k, 1024 q] = K_h Q_h^T   (PE, contraction=64)
    attn^T = exp(S^T / 8)                  (ScalarE, PSUM->SBUF, bf16)
    O'[65, q] += [V_h | 1]^T @ attn^T      (PE, accumulate over 16 k-chunks)
  row 64 of O' = softmax denominators; normalize rows 0..63 via DVE multiply
  with a GpSimd partition-broadcast reciprocal -> O^T [512, 2048]
  partial = O^T.T @ Wo-slice^T             (PE, accumulate over 4 chunks)
Q-projection chunks are emitted inside the attention head loop (own PSUM
tag) so the ScalarE-bound attention phase starts ~30us earlier.
"""

import sys

if "/opt/trn_rl_repo" not in sys.path:
    sys.path.insert(0, "/opt/trn_rl_repo")

import numpy as np
import ml_dtypes

P = 128
S = 2048
DIN = 1024
DG = 512          # per-core projection width (8 heads * 64)
HD = 64
NH_LOCAL = 8      # heads per core
N_CORES = 8
VA = NH_LOCAL * (HD + 1)  # V_aug free width (520)

_CACHE: dict = {}


def build_bass(repeat: int = 1):
    """Build the SPMD single-core program (same program on all 8 cores)."""
    from concourse import bacc, tile, mybir

    f32 = mybir.dt.float32
    bf16 = mybir.dt.bfloat16

    nc = bacc.Bacc("TRN2", target_bir_lowering=False, debug=False,
                   num_devices=N_CORES)

    xqT = nc.dram_tensor("xqT", [DIN, S], bf16, kind="ExternalInput")
    xkT = nc.dram_tensor("xkT", [DIN, S], bf16, kind="ExternalInput")
    xvT = nc.dram_tensor("xvT", [DIN, S], bf16, kind="ExternalInput")
    wqT = nc.dram_tensor("wqT", [DIN, DG], bf16, kind="ExternalInput")
    wkT = nc.dram_tensor("wkT", [DIN, DG], bf16, kind="ExternalInput")
    wvT = nc.dram_tensor("wvT", [DIN, DG], bf16, kind="ExternalInput")
    woT = nc.dram_tensor("woT", [DG, DIN], bf16, kind="ExternalInput")
    bqd = nc.dram_tensor("bq", [P, 4], f32, kind="ExternalInput")
    bkd = nc.dram_tensor("bk", [P, 4], f32, kind="ExternalInput")
    bvd = nc.dram_tensor("bv", [1, DG], f32, kind="ExternalInput")
    outp = nc.dram_tensor("outp", [S, DIN], f32, kind="ExternalOutput")

    with tile.TileContext(nc) as tc:
        for _ in range(repeat):
            _emit(nc, tc, xqT, xkT, xvT, wqT, wkT, wvT, woT, bqd, bkd, bvd,
                  outp)
    nc.compile()
    return nc


def _emit(nc, tc, xqT, xkT, xvT, wqT, wkT, wvT, woT, bqd, bkd, bvd, outp):
    from concourse import mybir
    from concourse.masks import make_identity

    f32 = mybir.dt.float32
    bf16 = mybir.dt.bfloat16
    Exp = mybir.ActivationFunctionType.Exp
    Copy = mybir.ActivationFunctionType.Copy
    mult = mybir.AluOpType.mult
    add_op = mybir.AluOpType.add

    with (
        tc.tile_pool(name="consts", bufs=1) as consts,
        tc.tile_pool(name="xin", bufs=3) as xin,
        tc.tile_pool(name="qkv", bufs=1) as qkvp,
        tc.tile_pool(name="attn", bufs=4) as attnp,
        tc.tile_pool(name="small", bufs=2) as smallp,
        tc.tile_pool(name="osb", bufs=3) as osbp,
        tc.tile_pool(name="ps", bufs=2, space="PSUM") as psp,
        tc.tile_pool(name="pav", bufs=2, space="PSUM") as pav,
    ):
        # PSUM budget (8 banks): "s" tag [128,1024]f32 x2 = 4 banks (scores),
        # "av" tag [65,512]f32 x2 = 2 banks (attention accumulators),
        # "qp" tag [128,512]f32 x2 = 2 banks (projections + out-proj), so
        # interleaved projection groups never block the score pipeline.
        QT = qkvp.tile([P, 4, S], bf16, tag="QT")
        KT = qkvp.tile([P, 4, S], bf16, tag="KT")
        vaug = qkvp.tile([P, 16, VA], bf16, tag="vaug")
        OT = qkvp.tile([P, 4, S], bf16, tag="OT")

        # ones columns of V_aug
        for h in range(NH_LOCAL):
            nc.vector.memset(vaug[:, :, h * (HD + 1) + HD], 1.0)

        def load_x(xdram, tag):
            # x^T [DIN, S] -> two SBUF tiles [P, 4, S]; each half split over
            # two DMA queues (sync + gpsimd) to halve load latency.
            xt = xdram.ap().rearrange("(h c p) m -> h p c m", h=2, p=P)
            halves = []
            for hhalf in range(2):
                xh = xin.tile([P, 4, S], bf16, tag="x")
                nc.sync.dma_start(xh[:, 0:2], xt[hhalf, :, 0:2])
                nc.gpsimd.dma_start(xh[:, 2:4], xt[hhalf, :, 2:4])
                halves.append(xh)
            return halves

        def proj_inputs(xdram, wdram, bdram, wtag):
            # Weight/bias/x DMAs are emitted here, per projection, so the
            # first matmul is not gated on later projections' loads.
            bias = consts.tile([P, 4], f32, tag=f"b_{wtag}")
            nc.sync.dma_start(bias[:], bdram.ap())
            w = consts.tile([P, 8, DG], bf16, tag=f"w_{wtag}")
            nc.sync.dma_start(w[:], wdram.ap().rearrange("(c p) m -> p c m",
                                                         p=P))
            halves = load_x(xdram, wtag)
            return w, bias, halves

        def proj_group(w, bias, halves, dstT, c, st):
            # dstT[dq, s] for dq chunk c, s-tile st (one PSUM group)
            pt = psp.tile([P, 512], f32, tag="qp", name=f"pj_{c}_{st}")
            for kc in range(8):
                nc.tensor.matmul(
                    pt[:],
                    w[:, kc, c * P:(c + 1) * P],
                    halves[kc // 4][:, kc % 4, st * 512:(st + 1) * 512],
                    start=(kc == 0),
                    stop=(kc == 7),
                )
            nc.vector.tensor_scalar_add(
                dstT[:, c, st * 512:(st + 1) * 512], pt[:],
                bias[:, c:c + 1])

        def proj_chunk(w, bias, halves, dstT, c):
            for st in range(4):
                proj_group(w, bias, halves, dstT, c, st)

        wk, bk, xkh = proj_inputs(xkT, wkT, bkd, "k")
        for c in range(4):
            proj_chunk(wk, bk, xkh, KT, c)

        # V projection inputs: V is computed directly in [s, dv] layout (x^T
        # chunks as the stationary operand) and drained straight into V_aug
        # columns with the bias added via a partition-broadcast row. The 16
        # V groups are emitted just-in-time inside head 0's k-loop.
        bvrow = consts.tile([1, DG], f32, tag="bvrow")
        nc.sync.dma_start(bvrow[:], bvd.ap())
        bvb = consts.tile([P, DG], f32, tag="bvb")
        nc.gpsimd.partition_broadcast(bvb[:], bvrow[:])
        bvb3 = bvb[:].rearrange("p (h f) -> p h f", f=HD)
        wv = consts.tile([P, 8, DG], bf16, tag="w_v")
        nc.sync.dma_start(wv[:], wvT.ap().rearrange("(c p) m -> p c m", p=P))
        xvh = load_x(xvT, "v")
        for sc in range(16):
            pt = psp.tile([P, DG], f32, tag="qp", name=f"pv_{sc}")
            for kc in range(8):
                nc.tensor.matmul(
                    pt[:],
                    xvh[kc // 4][:, kc % 4, sc * P:(sc + 1) * P],
                    wv[:, kc, :],
                    start=(kc == 0), stop=(kc == 7),
                )
            dst3 = vaug[:, sc].rearrange("p (h f) -> p h f", f=HD + 1)[:, :,
                                                                      0:HD]
            src3 = pt[:].rearrange("p (h f) -> p h f", f=HD)
            nc.vector.tensor_tensor(dst3, src3, bvb3, add_op)

        wq, bq, xqh = proj_inputs(xqT, wqT, bqd, "q")

        wo = consts.tile([P, 4, DIN], bf16, tag="wo")
        nc.sync.dma_start(wo[:], woT.ap().rearrange("(c p) m -> p c m", p=P))

        # attention, head by head; exp runs on 1024-wide tiles (2 q-tiles)
        # to amortize ScalarE per-instruction overhead. Q-projection chunks
        # are emitted just before the head pair that reads them, so the
        # PSUM "s"-tag slot FIFO interleaves Q projection with attention
        # and the ScalarE-bound phase starts ~30us earlier.
        qint = 0  # rolling pointer into Q-chunk groups 1..3 (12 groups)
        # half-outer: all heads' q-half 0 first, then q-half 1 — so the
        # out-projection of s-tiles 0..7 (which needs every head's half 0)
        # can start mid-attention instead of after the last head.
        for hh in range(2 * NH_LOCAL):
            half, h = hh // NH_LOCAL, hh % NH_LOCAL
            cq = h // 2
            off = (h % 2) * HD
            if hh == 0:
                proj_chunk(wq, bq, xqh, QT, 0)
            if True:
                avs = [pav.tile([HD + 1, 512], f32, tag="av",
                                name=f"av_{h}_{half}_{i}") for i in range(2)]
                for kc in range(16):
                    # stream the rest of the projections underneath the
                    # ScalarE-bound attention phase (own PSUM tag, so they
                    # never block the score pipeline):
                    # - V group sc lands just before head 0's AV reads it
                    # - K chunk c+1 projects during head c (needed by 2c+2)
                    # - Q chunk 1+i/4 projects during heads 1..3
                    if kc % 8 == 4 and qint < 12 and h >= 1:
                        proj_group(wq, bq, xqh, QT, 1 + qint // 4, qint % 4)
                        qint += 1
                    st_ = psp.tile([P, 1024], f32, tag="s")
                    for j in range(2):
                        qt = 2 * half + j
                        nc.tensor.matmul(
                            st_[:, j * 512:(j + 1) * 512],
                            KT[off:off + HD, cq, kc * P:(kc + 1) * P],
                            QT[off:off + HD, cq, qt * 512:(qt + 1) * 512],
                            start=True, stop=True)
                    at = attnp.tile([P, 1024], bf16, tag="at")
                    nc.scalar.activation(at[:], st_[:], Exp, scale=0.125)
                    for j in range(2):
                        nc.tensor.matmul(
                            avs[j][:],
                            vaug[:, kc, h * (HD + 1):(h + 1) * (HD + 1)],
                            at[:, j * 512:(j + 1) * 512],
                            start=(kc == 0), stop=(kc == 15))
                for j in range(2):
                    qt = 2 * half + j
                    rc = smallp.tile([1, 512], f32, tag="rc")
                    nc.vector.reciprocal(rc[:], avs[j][HD:HD + 1, :])
                    bc = smallp.tile([HD, 512], f32, tag="bc")
                    nc.gpsimd.partition_broadcast(bc[:], rc[0:1, :])
                    nc.vector.tensor_tensor(
                        OT[off:off + HD, cq, qt * 512:(qt + 1) * 512],
                        avs[j][0:HD, :], bc[:], mult)

        # out projection: partial[s, dout] = sum_dq OT[dq, s] * woT[dq, dout]
        for st in range(16):
            for nh in range(2):
                po = psp.tile([P, 512], f32, tag="qp", name=f"po_{st}_{nh}")
                for c in range(4):
                    nc.tensor.matmul(
                        po[:],
                        OT[:, c, st * P:(st + 1) * P],
                        wo[:, c, nh * 512:(nh + 1) * 512],
                        start=(c == 0), stop=(c == 3))
                ob = osbp.tile([P, 512], f32, tag="ob")
                nc.vector.tensor_copy(ob[:], po[:])
                nc.sync.dma_start(
                    outp.ap()[st * P:(st + 1) * P, nh * 512:(nh + 1) * 512],
                    ob[:])


def make_in_maps(q, k, v, Wq, bq, Wk, bk, Wv, bv, Wo, bo):
    bf = ml_dtypes.bfloat16
    in_maps = []
    for c in range(N_CORES):
        b_, g = c // 2, c % 2
        sl = slice(g * DG, (g + 1) * DG)
        in_maps.append({
            "xqT": np.ascontiguousarray(q[b_].T).astype(bf),
            "xkT": np.ascontiguousarray(k[b_].T).astype(bf),
            "xvT": np.ascontiguousarray(v[b_].T).astype(bf),
            "wqT": np.ascontiguousarray(Wq[sl].T).astype(bf),
            "wkT": np.ascontiguousarray(Wk[sl].T).astype(bf),
            "wvT": np.ascontiguousarray(Wv[sl].T).astype(bf),
            "woT": np.ascontiguousarray(Wo[:, sl].T).astype(bf),
            "bq": np.ascontiguousarray(
                bq[sl].astype(np.float32).reshape(4, P).T),
            "bk": np.ascontiguousarray(
                bk[sl].astype(np.float32).reshape(4, P).T),
            "bv": np.ascontiguousarray(
                bv[sl].astype(np.float32).reshape(1, DG)),
        })
    return in_maps


def assemble(results, bo):
    out = np.zeros((4, S, DIN), np.float32)
    for b_ in range(4):
        out[b_] = results[2 * b_]["outp"] + results[2 * b_ + 1]["outp"]
    out += np.asarray(bo, np.float32)[None, None, :]
    return out


def kernel(q, k, v, Wq, bq, Wk, bk, Wv, bv, Wo, bo):
    from concourse.bass_utils import run_bass_kernel_spmd

    if "nc" not in _CACHE:
        _CACHE["nc"] = build_bass()
    nc = _CACHE["nc"]
    in_maps = make_in_maps(q, k, v, Wq, bq, Wk, bk, Wv, bv, Wo, bo)
    res = run_bass_kernel_spmd(nc, in_maps, core_ids=list(range(N_CORES)))
    return assemble(res.results, bo)



# revision 1
# speedup vs baseline: 2.5054x; 2.5054x over previous
"""Multihead attention (B=4, S=2048, D=1024, H=16) on 8 Trainium2 NeuronCores.

Sharding: data-parallel over batch (4) x tensor-parallel over heads (2 groups
of 8 heads). Core c handles batch c//2, head-group c%2. Q/K/V projections are
column-parallel (each core owns 512 rows of Wq/Wk/Wv), attention is fully
local per head, out-projection is row-parallel (each core owns 512 columns of
Wo) producing a partial [S, D] output; the two partials per batch are summed
on the host (the "all-reduce").

Device layout (per core, all bf16 unless noted):
  xqT/xkT/xvT [1024, 2048]  = x[b].T            (host-transposed, bf16)
  wqT/wkT/wvT [1024, 512]   = W[g*512:,:].T
  woT         [512, 1024]   = Wo[:, g*512:].T
  bq/bk [128, 4] f32 (partition-major), bv [1, 512] f32
  outp        [2048, 1024] f32 (partial output)

On-chip pipeline (K proj -> V proj -> attention with Q proj interleaved):
  QT/KT [512, 2048] = W^T-slice .T @ x^T   (PE, accumulate over 8 K-chunks)
  V_aug [2048, 8*65]: V projected directly in [s, dv] layout (x^T chunks as
  the stationary operand) + a ones column per head
  per head h, per q-half (2 x 512 q):
    S^T tile [128 k, 1024 q] = K_h Q_h^T   (PE, contraction=64)
    attn^T = exp(S^T / 8)                  (ScalarE, PSUM->SBUF, bf16)
    O'[65, q] += [V_h | 1]^T @ attn^T      (PE, accumulate over 16 k-chunks)
  row 64 of O' = softmax denominators; normalize rows 0..63 via DVE multiply
  with a GpSimd partition-broadcast reciprocal -> O^T [512, 2048]
  partial = O^T.T @ Wo-slice^T             (PE, accumulate over 4 chunks)
Q-projection chunks are emitted inside the attention head loop (own PSUM
tag) so the ScalarE-bound attention phase starts ~30us earlier.
"""

import sys

if "/opt/trn_rl_repo" not in sys.path:
    sys.path.insert(0, "/opt/trn_rl_repo")

import numpy as np
import ml_dtypes

P = 128
S = 2048
DIN = 1024
DG = 512          # per-core projection width (8 heads * 64)
HD = 64
NH_LOCAL = 8      # heads per core
N_CORES = 8
VA = NH_LOCAL * (HD + 1)  # V_aug free width (520)

_CACHE: dict = {}


def build_bass(repeat: int = 1):
    """Build the SPMD single-core program (same program on all 8 cores)."""
    from concourse import bacc, tile, mybir

    f32 = mybir.dt.float32
    bf16 = mybir.dt.bfloat16

    nc = bacc.Bacc("TRN2", target_bir_lowering=False, debug=False,
                   num_devices=N_CORES)

    xqT = nc.dram_tensor("xqT", [DIN, S], bf16, kind="ExternalInput")
    xkT = nc.dram_tensor("xkT", [DIN, S], bf16, kind="ExternalInput")
    xvT = nc.dram_tensor("xvT", [DIN, S], bf16, kind="ExternalInput")
    wqT = nc.dram_tensor("wqT", [DIN, DG], bf16, kind="ExternalInput")
    wkT = nc.dram_tensor("wkT", [DIN, DG], bf16, kind="ExternalInput")
    wvT = nc.dram_tensor("wvT", [DIN, DG], bf16, kind="ExternalInput")
    woT = nc.dram_tensor("woT", [DG, DIN], bf16, kind="ExternalInput")
    bqd = nc.dram_tensor("bq", [P, 4], f32, kind="ExternalInput")
    bkd = nc.dram_tensor("bk", [P, 4], f32, kind="ExternalInput")
    bvd = nc.dram_tensor("bv", [1, DG], f32, kind="ExternalInput")
    outp = nc.dram_tensor("outp", [S, DIN], f32, kind="ExternalOutput")

    with tile.TileContext(nc) as tc:
        for _ in range(repeat):
            _emit(nc, tc, xqT, xkT, xvT, wqT, wkT, wvT, woT, bqd, bkd, bvd,
                  outp)
    nc.compile()
    return nc


def _emit(nc, tc, xqT, xkT, xvT, wqT, wkT, wvT, woT, bqd, bkd, bvd, outp):
    from concourse import mybir
    from concourse.masks import make_identity

    f32 = mybir.dt.float32
    bf16 = mybir.dt.bfloat16
    Exp = mybir.ActivationFunctionType.Exp
    Copy = mybir.ActivationFunctionType.Copy
    mult = mybir.AluOpType.mult
    add_op = mybir.AluOpType.add

    with (
        tc.tile_pool(name="consts", bufs=1) as consts,
        tc.tile_pool(name="xin", bufs=3) as xin,
        tc.tile_pool(name="qkv", bufs=1) as qkvp,
        tc.tile_pool(name="attn", bufs=4) as attnp,
        tc.tile_pool(name="small", bufs=2) as smallp,
        tc.tile_pool(name="osb", bufs=3) as osbp,
        tc.tile_pool(name="ps", bufs=2, space="PSUM") as psp,
        tc.tile_pool(name="pav", bufs=2, space="PSUM") as pav,
    ):
        # PSUM budget (8 banks): "s" tag [128,1024]f32 x2 = 4 banks (scores),
        # "av" tag [65,512]f32 x2 = 2 banks (attention accumulators),
        # "qp" tag [128,512]f32 x2 = 2 banks (projections + out-proj), so
        # interleaved projection groups never block the score pipeline.
        QT = qkvp.tile([P, 4, S], bf16, tag="QT")
        KT = qkvp.tile([P, 4, S], bf16, tag="KT")
        vaug = qkvp.tile([P, 16, VA], bf16, tag="vaug")
        OT = qkvp.tile([P, 4, S], bf16, tag="OT")

        # ones columns of V_aug
        for h in range(NH_LOCAL):
            nc.vector.memset(vaug[:, :, h * (HD + 1) + HD], 1.0)

        def load_x(xdram, tag):
            # x^T [DIN, S] -> two SBUF tiles [P, 4, S]; each half split over
            # two DMA queues (sync + gpsimd) to halve load latency.
            xt = xdram.ap().rearrange("(h c p) m -> h p c m", h=2, p=P)
            halves = []
            for hhalf in range(2):
                xh = xin.tile([P, 4, S], bf16, tag="x")
                nc.sync.dma_start(xh[:, 0:2], xt[hhalf, :, 0:2])
                nc.gpsimd.dma_start(xh[:, 2:4], xt[hhalf, :, 2:4])
                halves.append(xh)
            return halves

        def proj_inputs(xdram, wdram, bdram, wtag):
            # Weight/bias/x DMAs are emitted here, per projection, so the
            # first matmul is not gated on later projections' loads.
            bias = consts.tile([P, 4], f32, tag=f"b_{wtag}")
            nc.sync.dma_start(bias[:], bdram.ap())
            w = consts.tile([P, 8, DG], bf16, tag=f"w_{wtag}")
            nc.sync.dma_start(w[:], wdram.ap().rearrange("(c p) m -> p c m",
                                                         p=P))
            halves = load_x(xdram, wtag)
            return w, bias, halves

        def proj_group(w, bias, halves, dstT, c, st):
            # dstT[dq, s] for dq chunk c, s-tile st (one PSUM group)
            pt = psp.tile([P, 512], f32, tag="qp", name=f"pj_{c}_{st}")
            for kc in range(8):
                nc.tensor.matmul(
                    pt[:],
                    w[:, kc, c * P:(c + 1) * P],
                    halves[kc // 4][:, kc % 4, st * 512:(st + 1) * 512],
                    start=(kc == 0),
                    stop=(kc == 7),
                )
            nc.vector.tensor_scalar_add(
                dstT[:, c, st * 512:(st + 1) * 512], pt[:],
                bias[:, c:c + 1])

        def proj_chunk(w, bias, halves, dstT, c):
            for st in range(4):
                proj_group(w, bias, halves, dstT, c, st)

        wk, bk, xkh = proj_inputs(xkT, wkT, bkd, "k")
        for c in range(4):
            proj_chunk(wk, bk, xkh, KT, c)

        # V projection inputs: V is computed directly in [s, dv] layout (x^T
        # chunks as the stationary operand) and drained straight into V_aug
        # columns with the bias added via a partition-broadcast row. The 16
        # V groups are emitted just-in-time inside head 0's k-loop.
        bvrow = consts.tile([1, DG], f32, tag="bvrow")
        nc.sync.dma_start(bvrow[:], bvd.ap())
        bvb = consts.tile([P, DG], f32, tag="bvb")
        nc.gpsimd.partition_broadcast(bvb[:], bvrow[:])
        bvb3 = bvb[:].rearrange("p (h f) -> p h f", f=HD)
        wv = consts.tile([P, 8, DG], bf16, tag="w_v")
        nc.sync.dma_start(wv[:], wvT.ap().rearrange("(c p) m -> p c m", p=P))
        xvh = load_x(xvT, "v")
        for sc in range(16):
            pt = psp.tile([P, DG], f32, tag="qp", name=f"pv_{sc}")
            for kc in range(8):
                nc.tensor.matmul(
                    pt[:],
                    xvh[kc // 4][:, kc % 4, sc * P:(sc + 1) * P],
                    wv[:, kc, :],
                    start=(kc == 0), stop=(kc == 7),
                )
            dst3 = vaug[:, sc].rearrange("p (h f) -> p h f", f=HD + 1)[:, :,
                                                                      0:HD]
            src3 = pt[:].rearrange("p (h f) -> p h f", f=HD)
            nc.vector.tensor_tensor(dst3, src3, bvb3, add_op)

        wq, bq, xqh = proj_inputs(xqT, wqT, bqd, "q")

        wo = consts.tile([P, 4, DIN], bf16, tag="wo")
        nc.sync.dma_start(wo[:], woT.ap().rearrange("(c p) m -> p c m", p=P))

        # attention, head by head; exp runs on 1024-wide tiles (2 q-tiles)
        # to amortize ScalarE per-instruction overhead. Q-projection chunks
        # are emitted just before the head pair that reads them, so the
        # PSUM "s"-tag slot FIFO interleaves Q projection with attention
        # and the ScalarE-bound phase starts ~30us earlier.
        qint = 0  # rolling pointer into Q-chunk groups 1..3 (12 groups)
        # half-outer: all heads' q-half 0 first, then q-half 1 — so the
        # out-projection of s-tiles 0..7 (which needs every head's half 0)
        # can start mid-attention instead of after the last head.
        for hh in range(2 * NH_LOCAL):
            half, h = hh // NH_LOCAL, hh % NH_LOCAL
            cq = h // 2
            off = (h % 2) * HD
            if hh == 0:
                proj_chunk(wq, bq, xqh, QT, 0)
            if True:
                avs = [pav.tile([HD + 1, 512], f32, tag="av",
                                name=f"av_{h}_{half}_{i}") for i in range(2)]
                for kc in range(16):
                    # stream the rest of the projections underneath the
                    # ScalarE-bound attention phase (own PSUM tag, so they
                    # never block the score pipeline):
                    # - V group sc lands just before head 0's AV reads it
                    # - K chunk c+1 projects during head c (needed by 2c+2)
                    # - Q chunk 1+i/4 projects during heads 1..3
                    if kc % 8 == 4 and qint < 12 and h >= 1:
                        proj_group(wq, bq, xqh, QT, 1 + qint // 4, qint % 4)
                        qint += 1
                    st_ = psp.tile([P, 1024], f32, tag="s")
                    for j in range(2):
                        qt = 2 * half + j
                        nc.tensor.matmul(
                            st_[:, j * 512:(j + 1) * 512],
                            KT[off:off + HD, cq, kc * P:(kc + 1) * P],
                            QT[off:off + HD, cq, qt * 512:(qt + 1) * 512],
                            start=True, stop=True)
                    at = attnp.tile([P, 1024], bf16, tag="at")
                    nc.scalar.activation(at[:], st_[:], Exp, scale=0.125)
                    for j in range(2):
                        nc.tensor.matmul(
                            avs[j][:],
                            vaug[:, kc, h * (HD + 1):(h + 1) * (HD + 1)],
                            at[:, j * 512:(j + 1) * 512],
                            start=(kc == 0), stop=(kc == 15))
                for j in range(2):
                    qt = 2 * half + j
                    rc = smallp.tile([1, 512], f32, tag="rc")
                    nc.vector.reciprocal(rc[:], avs[j][HD:HD + 1, :])
                    bc = smallp.tile([HD, 512], f32, tag="bc")
                    nc.gpsimd.partition_broadcast(bc[:], rc[0:1, :])
                    nc.vector.tensor_tensor(
                        OT[off:off + HD, cq, qt * 512:(qt + 1) * 512],
                        avs[j][0:HD, :], bc[:], mult)

        # out projection: partial[s, dout] = sum_dq OT[dq, s] * woT[dq, dout]
        for st in range(16):
            for nh in range(2):
                po = psp.tile([P, 512], f32, tag="qp", name=f"po_{st}_{nh}")
                for c in range(4):
                    nc.tensor.matmul(
                        po[:],
                        OT[:, c, st * P:(st + 1) * P],
                        wo[:, c, nh * 512:(nh + 1) * 512],
                        start=(c == 0), stop=(c == 3))
                ob = osbp.tile([P, 512], f32, tag="ob")
                nc.vector.tensor_copy(ob[:], po[:])
                nc.sync.dma_start(
                    outp.ap()[st * P:(st + 1) * P, nh * 512:(nh + 1) * 512],
                    ob[:])


def make_in_maps(q, k, v, Wq, bq, Wk, bk, Wv, bv, Wo, bo):
    bf = ml_dtypes.bfloat16
    in_maps = []
    for c in range(N_CORES):
        b_, g = c // 2, c % 2
        sl = slice(g * DG, (g + 1) * DG)
        in_maps.append({
            "xqT": np.ascontiguousarray(q[b_].T).astype(bf),
            "xkT": np.ascontiguousarray(k[b_].T).astype(bf),
            "xvT": np.ascontiguousarray(v[b_].T).astype(bf),
            "wqT": np.ascontiguousarray(Wq[sl].T).astype(bf),
            "wkT": np.ascontiguousarray(Wk[sl].T).astype(bf),
            "wvT": np.ascontiguousarray(Wv[sl].T).astype(bf),
            "woT": np.ascontiguousarray(Wo[:, sl].T).astype(bf),
            "bq": np.ascontiguousarray(
                bq[sl].astype(np.float32).reshape(4, P).T),
            "bk": np.ascontiguousarray(
                bk[sl].astype(np.float32).reshape(4, P).T),
            "bv": np.ascontiguousarray(
                bv[sl].astype(np.float32).reshape(1, DG)),
        })
    return in_maps


def assemble(results, bo):
    out = np.zeros((4, S, DIN), np.float32)
    for b_ in range(4):
        out[b_] = results[2 * b_]["outp"] + results[2 * b_ + 1]["outp"]
    out += np.asarray(bo, np.float32)[None, None, :]
    return out


def kernel(q, k, v, Wq, bq, Wk, bk, Wv, bv, Wo, bo):
    from concourse.bass_utils import run_bass_kernel_spmd

    if "nc" not in _CACHE:
        _CACHE["nc"] = build_bass()
    nc = _CACHE["nc"]
    in_maps = make_in_maps(q, k, v, Wq, bq, Wk, bk, Wv, bv, Wo, bo)
    res = run_bass_kernel_spmd(nc, in_maps, core_ids=list(range(N_CORES)))
    return assemble(res.results, bo)

